# revision 19
# baseline (speedup 1.0000x reference)
"""Position-attention layer (dense_transformer) for Trainium2, 8 NeuronCores.

Data-parallel over batch B=8: one batch element per core. Per core:
  q = relu(Wq8 @ x8 + bq)  [80, 4096] f32r  (fp8 DoubleRow projection)
  k = relu(Wk8 @ x8 + bk)  [80, 4096] f32r
  vT = relu(x8^T @ Wv8^T + bv) [4096, 512] -> fp8e4 (DoubleRow + DVE relu)
  S^T[j,i] = sum_c k[c,j] q[c,i]    (energy in f32r, j on partitions)
  P = exp(S^T - 8) -> fp8e4         (paired activations; e^-8 cancels in num/l)
  l[i] = sum_j P[j,i]               (fp8 DoubleRow ones-stationary matmuls)
  oT[c,i] = sum_j vT[j,c] P[j,i]    (PV with vT stationary -> output already
                                     transposed; no PE transposes needed)
  out[c,i] = gamma[c,i] * oT[c,i] * (1/l)[i] + x[c,i]   (DVE, rl broadcast)

Measured per-matmul cost on this toolchain is stream + stationary-load +
~50c fixed (loads are not hidden), so the structure minimizes matmul count:
DoubleRow (K=256/instr) everywhere fp8 operands allow, N=512 moving always.
Host pre-quantizes x/Wq/Wk/Wv to fp8e4 and gamma to bf16. Numerically
simulated end-to-end rel err ~2.1e-3 vs the 2e-2 gate.
"""

import sys

sys.path.insert(0, "/opt/trn_rl_repo")

import numpy as np

B, C, H, W = 8, 512, 64, 64
HW = H * W          # 4096
CQK = 80
NCORES = 8
IB = 512            # i-block size for the attention stage
NB = HW // IB       # 8 i-blocks
NJ = HW // 128      # 32 j-tiles
NJP = NJ // 2       # 16 j-pairs (DoubleRow: 2 j-tiles per matmul)
EXP_BIAS = -8.0     # exp(S-8): S in [0, ~12.8] -> exp in [3.4e-4, 121] (e4m3 max 240)

_STATE = {}


def build_program(loop_reps=None):
    """Build the per-core Bass program. If loop_reps is set, wrap the whole
    kernel body in a hardware For_i loop (used for timing benchmarks only)."""
    from contextlib import ExitStack

    import concourse.bass as bass  # noqa: F401
    import concourse.tile as tile
    from concourse import bacc, mybir

    f32 = mybir.dt.float32
    f32r = mybir.dt.float32r
    bf16 = mybir.dt.bfloat16
    fp8 = mybir.dt.float8e4
    Relu = mybir.ActivationFunctionType.Relu
    Exp = mybir.ActivationFunctionType.Exp
    DR = mybir.MatmulPerfMode.DoubleRow
    Mult = mybir.AluOpType.mult
    Add = mybir.AluOpType.add

    nc = bacc.Bacc("TRN2", target_bir_lowering=False, debug=False)
    x = nc.declare_dram_parameter("x", [C, HW], f32, isOutput=False)
    x8 = nc.declare_dram_parameter("x8", [C, HW], fp8, isOutput=False)
    wq8 = nc.declare_dram_parameter("wq8", [C, CQK], fp8, isOutput=False)
    wk8 = nc.declare_dram_parameter("wk8", [C, CQK], fp8, isOutput=False)
    wv8 = nc.declare_dram_parameter("wv8", [C, C], fp8, isOutput=False)
    bq = nc.declare_dram_parameter("bq", [CQK, 1], f32, isOutput=False)
    bk = nc.declare_dram_parameter("bk", [CQK, 1], f32, isOutput=False)
    bv = nc.declare_dram_parameter("bv", [1, C], f32, isOutput=False)
    gamma = nc.declare_dram_parameter("gamma", [C, HW], bf16, isOutput=False)
    onesr = nc.declare_dram_parameter("onesr", [1, 128], f32, isOutput=False)
    out = nc.declare_dram_parameter("out", [C, HW], f32, isOutput=True)

    def body(tc, ctx):
        persist = ctx.enter_context(tc.tile_pool(name="persist", bufs=1))
        x_sb = persist.tile([128, 4, HW], f32, tag="x")
        x8_sb = persist.tile([128, 4, HW], fp8, tag="x8")
        wq_sb = persist.tile([128, 4, CQK], fp8, tag="wq")
        wk_sb = persist.tile([128, 4, CQK], fp8, tag="wk")
        wv_sb = persist.tile([128, 4, C], fp8, tag="wv")
        bq_sb = persist.tile([CQK, 1], f32, tag="bq")
        bk_sb = persist.tile([CQK, 1], f32, tag="bk")
        bv_sb = persist.tile([1, C], f32r, tag="bv")
        onesr_sb = persist.tile([1, 128], f32r, tag="onesr")
        # M=128 ones columns: the l-matmul then replicates l across all 128
        # partitions, so 1/l is directly usable as a DVE tensor operand
        ones8_sb = persist.tile([128, 2, 128], fp8, tag="ones8")
        ebias_sb = persist.tile([128, 1], f32, tag="ebias")
        q_sb = persist.tile([CQK, HW], f32r, tag="q")
        k_sb = persist.tile([CQK, HW], f32r, tag="k")
        vT_sb = persist.tile([128, NJ, C], fp8, tag="vT")

        nc.sync.dma_start(
            out=wq_sb, in_=wq8[:, :].rearrange("(k p) m -> p k m", p=128)
        )
        nc.sync.dma_start(
            out=wk_sb, in_=wk8[:, :].rearrange("(k p) m -> p k m", p=128)
        )
        nc.sync.dma_start(out=bq_sb, in_=bq[:, :])
        nc.sync.dma_start(out=bk_sb, in_=bk[:, :])
        nc.sync.dma_start(
            out=wv_sb, in_=wv8[:, :].rearrange("(k p) m -> p k m", p=128)
        )
        nc.sync.dma_start(out=bv_sb, in_=bv[:, :].bitcast(f32r))
        nc.sync.dma_start(out=onesr_sb, in_=onesr[:, :].bitcast(f32r))
        nc.vector.memset(ones8_sb, 1.0)
        nc.vector.memset(ebias_sb, EXP_BIAS)
        # x8 by n-chunks so the q/k projections can trail the DMA
        x8_re = x8[:, :].rearrange("(k p) n -> p k n", p=128)
        for n in range(8):
            nc.sync.dma_start(
                out=x8_sb[:, :, n * 512 : (n + 1) * 512],
                in_=x8_re[:, :, n * 512 : (n + 1) * 512],
            )
        # f32 x for the residual add (needed only by the output stage)
        x_re = x[:, :].rearrange("(k p) n -> p k n", p=128)
        for n in range(8):
            nc.sync.dma_start(
                out=x_sb[:, :, n * 512 : (n + 1) * 512],
                in_=x_re[:, :, n * 512 : (n + 1) * 512],
            )

        expp = ctx.enter_context(tc.tile_pool(name="expp", bufs=2))
        rlp = ctx.enter_context(tc.tile_pool(name="rlp", bufs=2))
        gp = ctx.enter_context(tc.tile_pool(name="gp", bufs=3))
        otp = ctx.enter_context(tc.tile_pool(name="otp", bufs=6))
        ps_s = ctx.enter_context(tc.tile_pool(name="ps_s", bufs=2, space="PSUM"))

        def qk_proj(w_sb, b_sb, dst, n, pool):
            pq = pool.tile([CQK, 512], f32, tag="pq")
            for kp in range(2):
                nc.tensor.matmul(
                    pq,
                    w_sb[:, 2 * kp : 2 * kp + 2, :],
                    x8_sb[:, 2 * kp : 2 * kp + 2, n * 512 : (n + 1) * 512],
                    start=(kp == 0),
                    stop=(kp == 1),
                    perf_mode=DR,
                )
            nc.scalar.activation(
                out=dst[:, n * 512 : (n + 1) * 512],
                in_=pq,
                func=Relu,
                bias=b_sb,
                scale=1.0,
            )

        def s_exp(b, jp, expst):
            """Energy pair (2 j-tiles, f32r) + one paired exp -> fp8."""
            ps = ps_s.tile([128, 2, 512], f32, tag="s")
            for h in range(2):
                j = 2 * jp + h
                nc.tensor.matmul(
                    ps[:, h, :],
                    k_sb[:, j * 128 : (j + 1) * 128],
                    q_sb[:, b * IB : (b + 1) * IB],
                    start=True,
                    stop=True,
                )
            nc.scalar.activation(
                out=expst[:, 2 * jp : 2 * jp + 2, :],
                in_=ps,
                func=Exp,
                bias=ebias_sb,
                scale=1.0,
            )

        expst_tiles = {}

        def pv_block(b, ps_o, ps_l, s_chunks=None):
            """l + PV (vT stationary -> [c,i] output) + scale/gamma/residual.

            s_chunks: optional list of 4 callables issuing the NEXT block's
            S/exp quarters, interleaved between PV groups so the PE always
            has dependency-free work queued (avoids stalls/p-state dips)."""
            expst = expst_tiles.pop(b)
            if s_chunks:
                s_chunks[0]()
            pl = ps_l.tile([128, IB], f32, tag="l")
            for jp in range(NJP):
                nc.tensor.matmul(
                    pl,
                    ones8_sb,
                    expst[:, 2 * jp : 2 * jp + 2, :],
                    start=(jp == 0),
                    stop=(jp == NJP - 1),
                    perf_mode=DR,
                )
            rl_b = rlp.tile([128, IB], f32, tag="rl")
            nc.vector.reciprocal(rl_b, pl)
            for cc in range(4):
                po = ps_o.tile([128, IB], f32, tag="o")
                for jp in range(NJP):
                    nc.tensor.matmul(
                        po,
                        vT_sb[:, 2 * jp : 2 * jp + 2, cc * 128 : (cc + 1) * 128],
                        expst[:, 2 * jp : 2 * jp + 2, :],
                        start=(jp == 0),
                        stop=(jp == NJP - 1),
                        perf_mode=DR,
                    )
                if s_chunks and cc < 3:
                    s_chunks[cc + 1]()
                g = gp.tile([128, IB], bf16, tag="g")
                nc.sync.dma_start(
                    out=g,
                    in_=gamma[cc * 128 : (cc + 1) * 128, b * IB : (b + 1) * IB],
                )
                ot = otp.tile([128, IB], f32, tag="ot")
                nc.vector.tensor_tensor(out=ot, in0=po, in1=rl_b, op=Mult)
                nc.vector.tensor_tensor(out=ot, in0=ot, in1=g, op=Mult)
                nc.vector.tensor_tensor(
                    out=ot, in0=ot, in1=x_sb[:, cc, b * IB : (b + 1) * IB], op=Add
                )
                nc.sync.dma_start(
                    out=out[cc * 128 : (cc + 1) * 128, b * IB : (b + 1) * IB],
                    in_=ot,
                )

        # stage 1: k full, q block 0, then S(0)/exp(0), rest of q, then v
        with tc.tile_pool(name="ps1", bufs=2, space="PSUM") as ps1:
            for n in range(8):
                qk_proj(wk_sb, bk_sb, k_sb, n, ps1)
            qk_proj(wq_sb, bq_sb, q_sb, 0, ps1)

            expst = expp.tile([128, NJ, IB], fp8, tag="expst", name="expst0")
            expst_tiles[0] = expst
            for jp in range(NJP):
                s_exp(0, jp, expst)

            for n in range(1, 8):
                qk_proj(wq_sb, bq_sb, q_sb, n, ps1)

            # v projection (fp8 DoubleRow), relu+fp8-convert on DVE
            with tc.tile_pool(name="ps1v", bufs=2, space="PSUM") as ps1v:
                for j in range(NJ):
                    pv = ps1v.tile([128, C], f32, tag="pv")
                    for kp in range(2):
                        nc.tensor.matmul(
                            pv,
                            x8_sb[:, 2 * kp : 2 * kp + 2, j * 128 : (j + 1) * 128],
                            wv_sb[:, 2 * kp : 2 * kp + 2, :],
                            start=(kp == 0),
                            stop=False,
                            perf_mode=DR,
                        )
                    nc.tensor.matmul(pv, onesr_sb, bv_sb, start=False, stop=True)
                    nc.vector.tensor_scalar_max(vT_sb[:, j, :], pv, 0.0)

        # steady loop: S/exp for block b overlaps PV/out for block b-1
        with tc.tile_pool(name="ps_o", bufs=3, space="PSUM") as ps_o, tc.tile_pool(
            name="ps_l", bufs=1, space="PSUM"
        ) as ps_l:
            for b in range(1, NB + 1):
                s_chunks = None
                if b < NB:
                    expst = expp.tile([128, NJ, IB], fp8, tag="expst", name=f"expst{b}")
                    expst_tiles[b] = expst

                    def mk(bb, lo, hi, tile_):
                        def go():
                            for jp in range(lo, hi):
                                s_exp(bb, jp, tile_)

                        return go

                    s_chunks = [mk(b, q * 4, q * 4 + 4, expst) for q in range(4)]
                pv_block(b - 1, ps_o, ps_l, s_chunks)

    with tile.TileContext(nc) as tc:
        with ExitStack() as ctx:
            if loop_reps is None:
                body(tc, ctx)
            else:
                with tc.For_i(0, loop_reps, 1):
                    body(tc, ctx)
    nc.compile()
    return nc


def _prep_host_inputs(inputs):
    """Fold BN scales into weights, quantize, build per-core input maps."""
    import ml_dtypes

    e4 = ml_dtypes.float8_e4m3
    f = lambda a: np.ascontiguousarray(np.asarray(a, dtype=np.float32))
    x = f(inputs["x"]).reshape(B, C, HW)
    wq8 = f((np.asarray(inputs["sq"])[:, None] * np.asarray(inputs["Wq"])).T).astype(e4)
    wk8 = f((np.asarray(inputs["sk"])[:, None] * np.asarray(inputs["Wk"])).T).astype(e4)
    wv8 = f((np.asarray(inputs["sv"])[:, None] * np.asarray(inputs["Wv"])).T).astype(e4)
    shared = {
        "wq8": wq8,
        "wk8": wk8,
        "wv8": wv8,
        "bq": f(inputs["bq"]).reshape(CQK, 1),
        "bk": f(inputs["bk"]).reshape(CQK, 1),
        "bv": f(inputs["bv"]).reshape(1, C),
        "gamma": f(inputs["gamma"]).reshape(C, HW).astype(ml_dtypes.bfloat16),
        "onesr": np.ones((1, 128), np.float32),
    }
    return [dict(shared, x=x[i], x8=x[i].astype(e4)) for i in range(NCORES)]


def kernel(**inputs):
    from concourse.bass_utils import run_bass_kernel_spmd

    if "nc" not in _STATE:
        _STATE["nc"] = build_program()
    nc = _STATE["nc"]
    in_maps = _prep_host_inputs(inputs)
    res = run_bass_kernel_spmd(nc, in_maps, list(range(NCORES)))
    out = np.stack([res.results[i]["out"] for i in range(NCORES)])
    return out.reshape(B, C, H, W).astype(np.float32)


if __name__ == "__main__":
    rng = np.random.default_rng(0)
    demo = {
        "x": rng.standard_normal((B, C, H, W), dtype=np.float32),
        "Wq": rng.standard_normal((CQK, C), dtype=np.float32) * 0.02,
        "Wk": rng.standard_normal((CQK, C), dtype=np.float32) * 0.02,
        "Wv": rng.standard_normal((C, C), dtype=np.float32) * 0.02,
        "sq": rng.uniform(0.5, 1.5, CQK).astype(np.float32),
        "bq": rng.standard_normal(CQK).astype(np.float32) * 0.1,
        "sk": rng.uniform(0.5, 1.5, CQK).astype(np.float32),
        "bk": rng.standard_normal(CQK).astype(np.float32) * 0.1,
        "sv": rng.uniform(0.5, 1.5, C).astype(np.float32),
        "bv": rng.standard_normal(C).astype(np.float32) * 0.1,
        "gamma": rng.standard_normal((C, H, W), dtype=np.float32) * 0.1,
    }
    y = kernel(**demo)
    print("kernel output:", y.shape, y.dtype, float(np.abs(y).max()))


# revision 20
# speedup vs baseline: 1.1504x; 1.1504x over previous
"""Position-attention layer (dense_transformer) for Trainium2, 8 NeuronCores.

Data-parallel over batch B=8: one batch element per core. Per core:
  q = relu(Wq8 @ x8 + bq)  [80, 4096] f32r  (fp8 DoubleRow projection)
  k = relu(Wk8 @ x8 + bk)  [80, 4096] f32r
  vT = relu(x8^T @ Wv8^T + bv) [4096, 512] -> fp8e4 (DoubleRow + DVE relu)
  S^T[j,i] = sum_c k[c,j] q[c,i]    (energy in f32r, j on partitions)
  P = exp(S^T - 8) -> fp8e4         (paired activations; e^-8 cancels in num/l)
  l[i] = sum_j P[j,i]               (fp8 DoubleRow ones-stationary matmuls)
  oT[c,i] = sum_j vT[j,c] P[j,i]    (PV with vT stationary -> output already
                                     transposed; no PE transposes needed)
  out[c,i] = gamma[c,i] * oT[c,i] * (1/l)[i] + x[c,i]   (DVE, rl broadcast)

Measured per-matmul cost on this toolchain is stream + stationary-load +
~50c fixed (loads are not hidden), so the structure minimizes matmul count:
DoubleRow (K=256/instr) everywhere fp8 operands allow, N=512 moving always.
Host pre-quantizes x/Wq/Wk/Wv to fp8e4 and gamma to bf16. Numerically
simulated end-to-end rel err ~2.1e-3 vs the 2e-2 gate.
"""

import sys

sys.path.insert(0, "/opt/trn_rl_repo")

import numpy as np

B, C, H, W = 8, 512, 64, 64
HW = H * W          # 4096
CQK = 80
NCORES = 8
IB = 512            # i-block size for the attention stage
NB = HW // IB       # 8 i-blocks
NJ = HW // 128      # 32 j-tiles
NJP = NJ // 2       # 16 j-pairs (DoubleRow: 2 j-tiles per matmul)
EXP_BIAS = -8.0     # exp(S-8): S in [0, ~12.8] -> exp in [3.4e-4, 121] (e4m3 max 240)

_STATE = {}


def build_program(loop_reps=None):
    """Build the per-core Bass program. If loop_reps is set, wrap the whole
    kernel body in a hardware For_i loop (used for timing benchmarks only)."""
    from contextlib import ExitStack

    import concourse.bass as bass  # noqa: F401
    import concourse.tile as tile
    from concourse import bacc, mybir

    f32 = mybir.dt.float32
    f32r = mybir.dt.float32r
    bf16 = mybir.dt.bfloat16
    fp8 = mybir.dt.float8e4
    Relu = mybir.ActivationFunctionType.Relu
    Exp = mybir.ActivationFunctionType.Exp
    DR = mybir.MatmulPerfMode.DoubleRow
    Mult = mybir.AluOpType.mult
    Add = mybir.AluOpType.add

    nc = bacc.Bacc("TRN2", target_bir_lowering=False, debug=False)
    x = nc.declare_dram_parameter("x", [C, HW], f32, isOutput=False)
    x8 = nc.declare_dram_parameter("x8", [C, HW], fp8, isOutput=False)
    wq8 = nc.declare_dram_parameter("wq8", [C, CQK], fp8, isOutput=False)
    wk8 = nc.declare_dram_parameter("wk8", [C, CQK], fp8, isOutput=False)
    wv8 = nc.declare_dram_parameter("wv8", [C, C], fp8, isOutput=False)
    bq = nc.declare_dram_parameter("bq", [CQK, 1], f32, isOutput=False)
    bk = nc.declare_dram_parameter("bk", [CQK, 1], f32, isOutput=False)
    bv = nc.declare_dram_parameter("bv", [1, C], f32, isOutput=False)
    gamma = nc.declare_dram_parameter("gamma", [C, HW], bf16, isOutput=False)
    onesr = nc.declare_dram_parameter("onesr", [1, 128], f32, isOutput=False)
    out = nc.declare_dram_parameter("out", [C, HW], f32, isOutput=True)

    def body(tc, ctx):
        persist = ctx.enter_context(tc.tile_pool(name="persist", bufs=1))
        x_sb = persist.tile([128, 4, HW], f32, tag="x")
        x8_sb = persist.tile([128, 4, HW], fp8, tag="x8")
        wq_sb = persist.tile([128, 4, CQK], fp8, tag="wq")
        wk_sb = persist.tile([128, 4, CQK], fp8, tag="wk")
        wv_sb = persist.tile([128, 4, C], fp8, tag="wv")
        bq_sb = persist.tile([CQK, 1], f32, tag="bq")
        bk_sb = persist.tile([CQK, 1], f32, tag="bk")
        bv_sb = persist.tile([1, C], f32r, tag="bv")
        onesr_sb = persist.tile([1, 128], f32r, tag="onesr")
        # M=128 ones columns: the l-matmul then replicates l across all 128
        # partitions, so 1/l is directly usable as a DVE tensor operand
        ones8_sb = persist.tile([128, 2, 128], fp8, tag="ones8")
        ebias_sb = persist.tile([128, 1], f32, tag="ebias")
        q_sb = persist.tile([CQK, HW], f32r, tag="q")
        k_sb = persist.tile([CQK, HW], f32r, tag="k")
        vT_sb = persist.tile([128, NJ, C], fp8, tag="vT")

        nc.sync.dma_start(
            out=wq_sb, in_=wq8[:, :].rearrange("(k p) m -> p k m", p=128)
        )
        nc.sync.dma_start(
            out=wk_sb, in_=wk8[:, :].rearrange("(k p) m -> p k m", p=128)
        )
        nc.sync.dma_start(out=bq_sb, in_=bq[:, :])
        nc.sync.dma_start(out=bk_sb, in_=bk[:, :])
        nc.sync.dma_start(
            out=wv_sb, in_=wv8[:, :].rearrange("(k p) m -> p k m", p=128)
        )
        nc.sync.dma_start(out=bv_sb, in_=bv[:, :].bitcast(f32r))
        nc.sync.dma_start(out=onesr_sb, in_=onesr[:, :].bitcast(f32r))
        nc.vector.memset(ones8_sb, 1.0)
        nc.vector.memset(ebias_sb, EXP_BIAS)
        # x8 by n-chunks so the q/k projections can trail the DMA
        x8_re = x8[:, :].rearrange("(k p) n -> p k n", p=128)
        for n in range(8):
            nc.sync.dma_start(
                out=x8_sb[:, :, n * 512 : (n + 1) * 512],
                in_=x8_re[:, :, n * 512 : (n + 1) * 512],
            )
        # f32 x for the residual add (needed only by the output stage)
        x_re = x[:, :].rearrange("(k p) n -> p k n", p=128)
        for n in range(8):
            nc.sync.dma_start(
                out=x_sb[:, :, n * 512 : (n + 1) * 512],
                in_=x_re[:, :, n * 512 : (n + 1) * 512],
            )

        expp = ctx.enter_context(tc.tile_pool(name="expp", bufs=2))
        rlp = ctx.enter_context(tc.tile_pool(name="rlp", bufs=2))
        gp = ctx.enter_context(tc.tile_pool(name="gp", bufs=3))
        otp = ctx.enter_context(tc.tile_pool(name="otp", bufs=6))
        ps_s = ctx.enter_context(tc.tile_pool(name="ps_s", bufs=2, space="PSUM"))

        def qk_proj(w_sb, b_sb, dst, n, pool):
            pq = pool.tile([CQK, 512], f32, tag="pq")
            for kp in range(2):
                nc.tensor.matmul(
                    pq,
                    w_sb[:, 2 * kp : 2 * kp + 2, :],
                    x8_sb[:, 2 * kp : 2 * kp + 2, n * 512 : (n + 1) * 512],
                    start=(kp == 0),
                    stop=(kp == 1),
                    perf_mode=DR,
                )
            nc.scalar.activation(
                out=dst[:, n * 512 : (n + 1) * 512],
                in_=pq,
                func=Relu,
                bias=b_sb,
                scale=1.0,
            )

        def s_exp(b, jp, expst):
            """Energy pair (2 j-tiles, f32r) + one paired exp -> fp8."""
            ps = ps_s.tile([128, 2, 512], f32, tag="s")
            for h in range(2):
                j = 2 * jp + h
                nc.tensor.matmul(
                    ps[:, h, :],
                    k_sb[:, j * 128 : (j + 1) * 128],
                    q_sb[:, b * IB : (b + 1) * IB],
                    start=True,
                    stop=True,
                )
            nc.scalar.activation(
                out=expst[:, 2 * jp : 2 * jp + 2, :],
                in_=ps,
                func=Exp,
                bias=ebias_sb,
                scale=1.0,
            )

        expst_tiles = {}

        def pv_block(b, ps_o, ps_l, s_chunks=None):
            """l + PV (vT stationary -> [c,i] output) + scale/gamma/residual.

            s_chunks: optional list of 4 callables issuing the NEXT block's
            S/exp quarters, interleaved between PV groups so the PE always
            has dependency-free work queued (avoids stalls/p-state dips)."""
            expst = expst_tiles.pop(b)
            if s_chunks:
                s_chunks[0]()
            pl = ps_l.tile([128, IB], f32, tag="l")
            for jp in range(NJP):
                nc.tensor.matmul(
                    pl,
                    ones8_sb,
                    expst[:, 2 * jp : 2 * jp + 2, :],
                    start=(jp == 0),
                    stop=(jp == NJP - 1),
                    perf_mode=DR,
                )
            rl_b = rlp.tile([128, IB], f32, tag="rl")
            # epsilon guards 1/l against rows whose fp8 exp underflows to 0
            # (can't happen for the calibrated input distribution, but free)
            nc.vector.tensor_scalar_add(rl_b, pl, 1e-20)
            nc.vector.reciprocal(rl_b, rl_b)
            for cc in range(4):
                po = ps_o.tile([128, IB], f32, tag="o")
                for jp in range(NJP):
                    nc.tensor.matmul(
                        po,
                        vT_sb[:, 2 * jp : 2 * jp + 2, cc * 128 : (cc + 1) * 128],
                        expst[:, 2 * jp : 2 * jp + 2, :],
                        start=(jp == 0),
                        stop=(jp == NJP - 1),
                        perf_mode=DR,
                    )
                if s_chunks and cc < 3:
                    s_chunks[cc + 1]()
                g = gp.tile([128, IB], bf16, tag="g")
                nc.sync.dma_start(
                    out=g,
                    in_=gamma[cc * 128 : (cc + 1) * 128, b * IB : (b + 1) * IB],
                )
                ot = otp.tile([128, IB], f32, tag="ot")
                nc.vector.tensor_tensor(out=ot, in0=po, in1=rl_b, op=Mult)
                nc.vector.tensor_tensor(out=ot, in0=ot, in1=g, op=Mult)
                nc.vector.tensor_tensor(
                    out=ot, in0=ot, in1=x_sb[:, cc, b * IB : (b + 1) * IB], op=Add
                )
                nc.sync.dma_start(
                    out=out[cc * 128 : (cc + 1) * 128, b * IB : (b + 1) * IB],
                    in_=ot,
                )

        # stage 1: k full, q block 0, then S(0)/exp(0), rest of q, then v
        with tc.tile_pool(name="ps1", bufs=2, space="PSUM") as ps1:
            for n in range(8):
                qk_proj(wk_sb, bk_sb, k_sb, n, ps1)
            qk_proj(wq_sb, bq_sb, q_sb, 0, ps1)

            expst = expp.tile([128, NJ, IB], fp8, tag="expst", name="expst0")
            expst_tiles[0] = expst
            for jp in range(NJP):
                s_exp(0, jp, expst)

            for n in range(1, 8):
                qk_proj(wq_sb, bq_sb, q_sb, n, ps1)

            # v projection (fp8 DoubleRow), relu+fp8-convert on DVE
            with tc.tile_pool(name="ps1v", bufs=2, space="PSUM") as ps1v:
                for j in range(NJ):
                    pv = ps1v.tile([128, C], f32, tag="pv")
                    for kp in range(2):
                        nc.tensor.matmul(
                            pv,
                            x8_sb[:, 2 * kp : 2 * kp + 2, j * 128 : (j + 1) * 128],
                            wv_sb[:, 2 * kp : 2 * kp + 2, :],
                            start=(kp == 0),
                            stop=False,
                            perf_mode=DR,
                        )
                    nc.tensor.matmul(pv, onesr_sb, bv_sb, start=False, stop=True)
                    nc.vector.tensor_scalar_max(vT_sb[:, j, :], pv, 0.0)

        # steady loop: S/exp for block b overlaps PV/out for block b-1
        with tc.tile_pool(name="ps_o", bufs=3, space="PSUM") as ps_o, tc.tile_pool(
            name="ps_l", bufs=1, space="PSUM"
        ) as ps_l:
            for b in range(1, NB + 1):
                s_chunks = None
                if b < NB:
                    expst = expp.tile([128, NJ, IB], fp8, tag="expst", name=f"expst{b}")
                    expst_tiles[b] = expst

                    def mk(bb, lo, hi, tile_):
                        def go():
                            for jp in range(lo, hi):
                                s_exp(bb, jp, tile_)

                        return go

                    s_chunks = [mk(b, q * 4, q * 4 + 4, expst) for q in range(4)]
                pv_block(b - 1, ps_o, ps_l, s_chunks)

    with tile.TileContext(nc) as tc:
        with ExitStack() as ctx:
            if loop_reps is None:
                body(tc, ctx)
            else:
                with tc.For_i(0, loop_reps, 1):
                    body(tc, ctx)
    nc.compile()
    return nc


def _prep_host_inputs(inputs):
    """Fold BN scales into weights, quantize, build per-core input maps."""
    import ml_dtypes

    e4 = ml_dtypes.float8_e4m3
    f = lambda a: np.ascontiguousarray(np.asarray(a, dtype=np.float32))
    x = f(inputs["x"]).reshape(B, C, HW)
    wq8 = f((np.asarray(inputs["sq"])[:, None] * np.asarray(inputs["Wq"])).T).astype(e4)
    wk8 = f((np.asarray(inputs["sk"])[:, None] * np.asarray(inputs["Wk"])).T).astype(e4)
    wv8 = f((np.asarray(inputs["sv"])[:, None] * np.asarray(inputs["Wv"])).T).astype(e4)
    shared = {
        "wq8": wq8,
        "wk8": wk8,
        "wv8": wv8,
        "bq": f(inputs["bq"]).reshape(CQK, 1),
        "bk": f(inputs["bk"]).reshape(CQK, 1),
        "bv": f(inputs["bv"]).reshape(1, C),
        "gamma": f(inputs["gamma"]).reshape(C, HW).astype(ml_dtypes.bfloat16),
        "onesr": np.ones((1, 128), np.float32),
    }
    return [dict(shared, x=x[i], x8=x[i].astype(e4)) for i in range(NCORES)]


def kernel(**inputs):
    from concourse.bass_utils import run_bass_kernel_spmd

    if "nc" not in _STATE:
        _STATE["nc"] = build_program()
    nc = _STATE["nc"]
    in_maps = _prep_host_inputs(inputs)
    res = run_bass_kernel_spmd(nc, in_maps, list(range(NCORES)))
    out = np.stack([res.results[i]["out"] for i in range(NCORES)])
    return out.reshape(B, C, H, W).astype(np.float32)


if __name__ == "__main__":
    rng = np.random.default_rng(0)
    demo = {
        "x": rng.standard_normal((B, C, H, W), dtype=np.float32),
        "Wq": rng.standard_normal((CQK, C), dtype=np.float32) * 0.02,
        "Wk": rng.standard_normal((CQK, C), dtype=np.float32) * 0.02,
        "Wv": rng.standard_normal((C, C), dtype=np.float32) * 0.02,
        "sq": rng.uniform(0.5, 1.5, CQK).astype(np.float32),
        "bq": rng.standard_normal(CQK).astype(np.float32) * 0.1,
        "sk": rng.uniform(0.5, 1.5, CQK).astype(np.float32),
        "bk": rng.standard_normal(CQK).astype(np.float32) * 0.1,
        "sv": rng.uniform(0.5, 1.5, C).astype(np.float32),
        "bv": rng.standard_normal(C).astype(np.float32) * 0.1,
        "gamma": rng.standard_normal((C, H, W), dtype=np.float32) * 0.1,
    }
    y = kernel(**demo)
    print("kernel output:", y.shape, y.dtype, float(np.abs(y).max()))
